# revision 1
# baseline (speedup 1.0000x reference)
"""FLA GatedDeltaNet layer on 8 Trainium2 NeuronCores.

Sharding: data-parallel over batch (2 groups) x tensor-parallel over heads
(4 shards of 2 heads). Each core computes its batch element with its 2 heads
end-to-end (projections, short conv, gated delta rule recurrence, gated
RMSNorm, o_proj partial). Host sums the 4 o_proj partials per batch element.

Recurrence uses the chunked WY form (chunk C=128):
  S_t = exp(g_t) S_{t-1};  u_t = beta_t (v_t - k_t^T S_t);  S_t += k_t u_t^T
  per chunk: (I + N) U = beta (V - Lam K S0),  N[t,i] = b_t e^{c_t-c_i} k_t.k_i
  TmT = transposed (I+N)^{-1} via product-form doubling (N nilpotent),
  O = Lam Q S0 + (QK^T . decay) U,  S1 = gam S0 + ((gam/Lam) K)^T U.

Dtypes: bf16 operands for all big matmuls (weights, activations), fp32 PSUM
accumulation, fp32r for the recurrent state S and the decay broadcast rhs.
"""
import sys

if "/opt/trn_rl_repo" not in sys.path:
    sys.path.insert(0, "/opt/trn_rl_repo")

import numpy as np
import ml_dtypes

import concourse.bass as bass
import concourse.bacc as bacc
import concourse.mybir as mybir
import concourse.tile as tile
from concourse.bass_utils import run_bass_kernel_spmd

F32 = mybir.dt.float32
F32R = mybir.dt.float32r
BF16 = mybir.dt.bfloat16
AF = mybir.ActivationFunctionType
OP = mybir.AluOpType

B, T, D = 2, 1024, 1024
H, DK, DV, KC = 8, 128, 256, 4
HL = 2              # heads per core
C = 128             # chunk length
NCH = T // C        # 8 chunks
NEG = -1e30
EPS = 1e-5
QSCALE = float(DK) ** -0.5

_cache = {}


def build_kernel():
    nc = bacc.Bacc(None, target_bir_lowering=False)

    xT = nc.dram_tensor("xT", [D, T], BF16, kind="ExternalInput")
    Wqk = nc.dram_tensor("Wqk", [D, 512], BF16, kind="ExternalInput")
    Wv = nc.dram_tensor("Wv", [D, 512], BF16, kind="ExternalInput")
    Wg = nc.dram_tensor("Wg", [D, 512], BF16, kind="ExternalInput")
    Wba = nc.dram_tensor("Wba", [D, 4], BF16, kind="ExternalInput")
    Wo = nc.dram_tensor("Wo", [HL * DV, D], BF16, kind="ExternalInput")
    convd = nc.dram_tensor("convd", [8, KC, 128, 128], BF16, kind="ExternalInput")
    adt = nc.dram_tensor("adt", [16, 2], F32, kind="ExternalInput")
    maskI = nc.dram_tensor("maskI", [128, 128], F32, kind="ExternalInput")
    maskS = nc.dram_tensor("maskS", [128, 128], F32, kind="ExternalInput")
    identg = nc.dram_tensor("identg", [16, 16], F32, kind="ExternalInput")
    identb = nc.dram_tensor("identb", [128, 128], BF16, kind="ExternalInput")
    outD = nc.dram_tensor("out", [T, D], BF16, kind="ExternalOutput")

    with tile.TileContext(nc, pool_alloc_mode="queue") as tc, \
         tc.tile_pool(name="res", bufs=1) as res:

        # ---------------- resident (small) loads ----------------
        Wo_s = res.tile([128, 4, D], BF16)
        Wba_s = res.tile([128, 8, 4], BF16)
        nc.sync.dma_start(out=Wba_s, in_=Wba.rearrange("(dt p) c -> p dt c", p=128))
        adt_s = res.tile([16, 2], F32)
        nc.sync.dma_start(out=adt_s, in_=adt[:, :])
        maskIS_s = res.tile([128, 256], F32)
        identg_s = res.tile([16, 16], F32)
        identb_s = res.tile([128, 128], BF16)
        nc.sync.dma_start(out=identb_s, in_=identb[:, :])
        cdt_all = res.tile([128, 8, KC, 128], BF16)

        ones_rowf = res.tile([1, 128], F32)
        nc.vector.memset(ones_rowf, 1.0)
        ones_row = res.tile([1, 128], F32R)
        nc.vector.tensor_copy(ones_row[:, :], ones_rowf[:, :])
        zerob_col = res.tile([128, 4], BF16)
        nc.vector.memset(zerob_col, 0.0)
        onesb_col = res.tile([128, 1], BF16)
        nc.vector.memset(onesb_col, 1.0)
        eps6_col = res.tile([128, 1], F32)
        nc.vector.memset(eps6_col, 1e-6)
        epsn_col = res.tile([128, 1], F32)
        nc.vector.memset(epsn_col, EPS)
        zeros16 = res.tile([16, 128], F32)
        nc.vector.memset(zeros16, 0.0)

        # persistent per-block activation tiles (k0 q0 k1 q1 block order)
        qk_all = res.tile([128, 4, T], BF16)
        vT_all = res.tile([128, 4, T], BF16)
        nsq_s = [res.tile([1, T], F32, tag=f"nsq{i}", name=f"nsq{i}")
                 for i in range(4)]
        ba_s = res.tile([4, T], F32)
        gvsnw = res.tile([128, NCH, 2 * DV], BF16)     # silu(gv), row-major
        og_s = res.tile([128, NCH, 2 * DV], BF16)      # o (later gated), row-major
        S_s = [[res.tile([128, DV], BF16, tag=f"S{hl}_{b}", name=f"S{hl}_{b}")
                for b in range(2)] for hl in range(HL)]
        ssq_all = res.tile([128, 16], F32)
        rstd_all = res.tile([128, 16], F32)

        # prep-chain tiles (tiny, resident so prep overlaps projections)
        bagB = res.tile([16, 128], F32)
        bagA = res.tile([16, 128], F32)
        bet_rows = res.tile([16, 128], F32)
        sg_rows = res.tile([16, 128], F32)
        g_rows = res.tile([16, 128], F32)
        c_rows = res.tile([16, 128], F32)
        lnb_rows = res.tile([16, 128], F32)
        cb_rows = res.tile([16, 128], F32)
        lam_rows = res.tile([16, 128], F32)
        lnq4 = [res.tile([1, T], F32, tag=f"lnq{i}", name=f"lnq{i}")
                for i in range(4)]
        lnm_rows = res.tile([16, 128], F32)
        lnn_rows = res.tile([16, 128], F32)
        cqs_rows = res.tile([16, 128], F32)
        cA_rows = res.tile([16, 128], F32)
        cN_rows = res.tile([16, 128], F32)
        ccn_rows = res.tile([16, 128], F32)
        cln_rows = res.tile([16, 128], F32)
        lamq_rows = res.tile([16, 128], F32)
        edn_rows = res.tile([16, 128], F32)
        lamn_rows = res.tile([16, 128], F32)
        nbl_rows = res.tile([16, 128], F32)
        ccb_flat = res.tile([1, 2, 16 * 128], F32R)
        cols_t = {nm: res.tile([128, 16], F32, tag=f"cols_{nm}", name=f"cols_{nm}")
                  for nm in ("ccn", "lam", "ed", "b", "nbl")}
        glast_row = res.tile([1, 16], F32R)
        gamb_s = res.tile([128, 16], F32)

        # ======== projection scope (xT/Wg freed afterwards) ========
        with tc.tile_pool(name="xp", bufs=1) as xp, \
             tc.tile_pool(name="wstream", bufs=3) as wstream, \
             tc.tile_pool(name="pre", bufs=2) as pre_pool, \
             tc.tile_pool(name="psJ", bufs=4, space="PSUM") as psJ, \
             tc.tile_pool(name="psn", bufs=2, space="PSUM") as psn, \
             tc.tile_pool(name="psP", bufs=2, space="PSUM") as psP:

            xT_s = [xp.tile([128, T], BF16, tag=f"xT{i}", name=f"xT{i}")
                    for i in range(8)]
            wt_pre = {}
            for blk in [2, 0]:
                wt_pre[blk] = wstream.tile([128, 8, 128], BF16, tag="w",
                                           name=f"w{blk}")
                nc.sync.dma_start(
                    out=wt_pre[blk],
                    in_=Wqk.rearrange("(dt p) c -> p dt c", p=128)
                    [:, :, blk * 128:(blk + 1) * 128])
            for dt_i in range(8):
                nc.sync.dma_start(
                    out=xT_s[dt_i],
                    in_=xT.rearrange("(dt p) t -> p dt t", p=128)[:, dt_i, :])
            Wg_s = xp.tile([128, 8, 512], BF16)
            nc.sync.dma_start(out=maskIS_s[:, 0:128], in_=maskI[:, :])
            nc.sync.dma_start(out=maskIS_s[:, 128:256], in_=maskS[:, :])
            nc.sync.dma_start(out=identg_s, in_=identg[:, :])

            # ---- transposed projections: ba first, then q,k,v c-blocks ----
            # block index: 0..3 = q0,q1,k0,k1 ; 4..7 = v halves ; 8 = ba
            dest_map = {0: qk_all[:, 1, :], 1: qk_all[:, 3, :],
                        2: qk_all[:, 0, :], 3: qk_all[:, 2, :]}
            for blk in [8, 2, 0, 3, 1, 4, 5, 6, 7]:
                nparts = 128 if blk < 8 else 4
                psums = [psJ.tile([nparts, 512], F32, tag="psJ", name=f"pj{blk}_{h}")
                         for h in range(2)]
                if blk < 8:
                    if blk in wt_pre:
                        wt8 = wt_pre[blk]
                    else:
                        wsrc = Wqk if blk < 4 else Wv
                        cb = blk if blk < 4 else blk - 4
                        wt8 = wstream.tile([128, 8, 128], BF16, tag="w",
                                           name=f"w{blk}")
                        nc.sync.dma_start(
                            out=wt8,
                            in_=wsrc.rearrange("(dt p) c -> p dt c", p=128)[:, :, cb * 128:(cb + 1) * 128])
                for d in range(8):
                    wt = wt8[:, d, :] if blk < 8 else Wba_s[:, d, :]
                    for half in range(2):
                        nc.tensor.matmul(
                            psums[half][:, :], wt,
                            xT_s[d][:, half * 512:(half + 1) * 512],
                            start=(d == 0), stop=(d == 7))
                if blk < 8:
                    pret = pre_pool.tile([128, T + 3], BF16, tag="pre", name=f"pre{blk}")
                    nc.vector.tensor_copy(pret[:, 0:3], zerob_col[:, 0:3])
                    nc.vector.tensor_copy(pret[:, 3:3 + 512], psums[0][:, :])
                    nc.scalar.activation(pret[:, 3 + 512:3 + 1024], psums[1][:, :],
                                         AF.Copy)
                    cdt = cdt_all[:, blk, :, :]
                    nc.sync.dma_start(
                        out=cdt,
                        in_=convd.rearrange("b k p m -> p b k m")[:, blk, :, :])
                    dest = vT_all[:, blk - 4, :] if blk >= 4 else dest_map[blk]
                    cps2 = [psJ.tile([128, 512], F32, tag="psJ", name=f"cv{blk}_{h}")
                            for h in range(2)]
                    for tap in range(KC):
                        for half in range(2):
                            nc.tensor.matmul(
                                cps2[half][:, :], cdt[:, tap, :],
                                pret[:, half * 512 + tap:half * 512 + tap + 512],
                                start=(tap == 0), stop=(tap == KC - 1))
                    for half in range(2):
                        nc.scalar.activation(dest[:, half * 512:(half + 1) * 512],
                                             cps2[half][:, :], AF.Silu)
                    if blk < 4:
                        # l2 norm stats only; normalization is folded into the
                        # decay scalars (exp-space) in the recurrence scope
                        sq = pre_pool.tile([128, T], BF16, tag="sq", name=f"sq{blk}")
                        nc.vector.tensor_mul(sq[:, :], dest[:, :], dest[:, :])
                        for q2 in range(2):
                            nsp = psn.tile([1, 512], F32, tag="psn", name=f"ns{blk}_{q2}")
                            nc.tensor.matmul(nsp[:, :], onesb_col[:, :],
                                             sq[:, q2 * 512:(q2 + 1) * 512],
                                             start=True, stop=True)
                            nc.vector.tensor_copy(
                                nsq_s[blk][:, q2 * 512:(q2 + 1) * 512], nsp[:, :])
                else:
                    for half in range(2):
                        nc.vector.tensor_copy(ba_s[:, half * 512:(half + 1) * 512],
                                              psums[half][:, :])

            # ---- gv projection (row-major) + silu ----
            for dt_i in range(8):
                nc.sync.dma_start(
                    out=Wg_s[:, dt_i, :],
                    in_=Wg.rearrange("(dt p) c -> p dt c", p=128)[:, dt_i, :])
            for ct_i in range(4):
                nc.sync.dma_start(
                    out=Wo_s[:, ct_i, :],
                    in_=Wo.rearrange("(ct p) d -> p ct d", p=128)[:, ct_i, :])
            for tt in range(NCH):
                gps = psJ.tile([128, 512], F32, tag="psJ", name=f"gv{tt}")
                for d in range(8):
                    nc.tensor.matmul(gps[:, :], xT_s[d][:, tt * 128:(tt + 1) * 128],
                                     Wg_s[:, d, :], start=(d == 0), stop=(d == 7))
                nc.scalar.activation(gvsnw[:, tt, :], gps[:, :], AF.Silu)

            # ---- per-chunk scalar streams (l2 norms folded into exp space) ----
            for f in range(2):
                nc.sync.dma_start(
                    out=bagB[f * NCH:(f + 1) * NCH, :],
                    in_=ba_s[f:f + 1, :].rearrange("o (j t) -> o j t", j=NCH))
                nc.sync.dma_start(
                    out=bagA[f * NCH:(f + 1) * NCH, :],
                    in_=ba_s[2 + f:3 + f, :].rearrange("o (j t) -> o j t", j=NCH))
            nc.scalar.activation(bet_rows[:, :], bagB[:, :], AF.Sigmoid)
            nc.scalar.activation(sg_rows[:, :], bagA[:, :], AF.Sigmoid,
                                 scale=-1.0, bias=adt_s[:, 0:1])
            # g = exp(A_log) * ln(sigmoid(-(a+dt_bias)))  [= -expA * softplus]
            nc.scalar.activation(g_rows[:, :], sg_rows[:, :], AF.Ln)
            nc.scalar.activation(g_rows[:, :], g_rows[:, :], AF.Copy,
                                 scale=adt_s[:, 1:2])
            nc.vector.tensor_tensor_scan(c_rows[:, :], g_rows[:, :], zeros16[:, :],
                                         0.0, op0=OP.add, op1=OP.add)
            nc.scalar.activation(lnb_rows[:, :], bet_rows[:, :], AF.Ln)
            nc.vector.tensor_tensor(out=cb_rows[:, :], in0=c_rows[:, :],
                                    in1=lnb_rows[:, :], op=OP.add)
            nc.scalar.activation(lam_rows[:, :], c_rows[:, :], AF.Exp)

            # half-log norms: ln|q|, ln|k| in chunk-row layout
            for i in range(4):
                nc.scalar.activation(lnq4[i][:, :], nsq_s[i][:, :], AF.Ln,
                                     bias=eps6_col[0:1, :])
            for f in range(2):
                nc.sync.dma_start(
                    out=lnm_rows[f * NCH:(f + 1) * NCH, :],
                    in_=lnq4[f][0:1, :].rearrange("o (j t) -> o j t", j=NCH))
                nc.sync.dma_start(
                    out=lnn_rows[f * NCH:(f + 1) * NCH, :],
                    in_=lnq4[2 + f][0:1, :].rearrange("o (j t) -> o j t", j=NCH))

            nc.vector.tensor_scalar_add(cqs_rows[:, :], c_rows[:, :],
                                        float(np.log(QSCALE)))
            nc.vector.scalar_tensor_tensor(
                out=cA_rows[:, :], in0=lnm_rows[:, :], scalar=-0.5,
                in1=cqs_rows[:, :], op0=OP.mult, op1=OP.add)
            nc.vector.scalar_tensor_tensor(
                out=cN_rows[:, :], in0=lnn_rows[:, :], scalar=-0.5,
                in1=cb_rows[:, :], op0=OP.mult, op1=OP.add)
            nc.vector.scalar_tensor_tensor(
                out=ccn_rows[:, :], in0=lnn_rows[:, :], scalar=0.5,
                in1=c_rows[:, :], op0=OP.mult, op1=OP.add)
            nc.vector.scalar_tensor_tensor(
                out=cln_rows[:, :], in0=lnn_rows[:, :], scalar=-0.5,
                in1=c_rows[:, :], op0=OP.mult, op1=OP.add)

            nc.scalar.activation(lamq_rows[:, :], cA_rows[:, :], AF.Exp)
            nc.scalar.activation(edn_rows[:, :], ccn_rows[:, :], AF.Exp,
                                 scale=-1.0, bias=c_rows[:, 127:128])
            nc.scalar.activation(lamn_rows[:, :], cln_rows[:, :], AF.Exp)
            nc.vector.tensor_tensor(out=nbl_rows[:, :], in0=bet_rows[:, :],
                                    in1=lamn_rows[:, :], op=OP.mult)
            nc.scalar.activation(nbl_rows[:, :], nbl_rows[:, :], AF.Copy, scale=-1.0)

            # flatten decay-col rows to single-partition tiles for PE rhs
            nc.gpsimd.dma_start(
                out=ccb_flat[0:1, 0, :].rearrange("p (r t) -> p r t", r=16),
                in_=cA_rows[:, :])
            nc.gpsimd.dma_start(
                out=ccb_flat[0:1, 1, :].rearrange("p (r t) -> p r t", r=16),
                in_=cN_rows[:, :])

            for nm, rt in [("ccn", ccn_rows), ("lam", lamq_rows), ("ed", edn_rows),
                           ("b", bet_rows), ("nbl", nbl_rows)]:
                ps = psP.tile([128, 16], F32, tag="psP", name=f"tc_{nm}")
                nc.tensor.transpose(ps[:, :], rt[:, :], identg_s[:, :])
                nc.vector.tensor_copy(cols_t[nm][:, :], ps[:, :])
            ccn_cols, lam_cols, ed_cols, b_cols, nbl_cols = (
                cols_t["ccn"], cols_t["lam"], cols_t["ed"], cols_t["b"],
                cols_t["nbl"])

            glast_ps = psP.tile([1, 16], F32, tag="psP", name="glast_ps")
            nc.tensor.transpose(glast_ps[:, :], lam_rows[:, 127:128],
                                identg_s[:, :])
            nc.vector.tensor_copy(glast_row[:, :], glast_ps[:, :])
            gamb_ps = psP.tile([128, 16], F32, tag="psP", name="gamb_ps")
            nc.tensor.matmul(gamb_ps[:, :], ones_row[:, :],
                             glast_row[:, :], start=True, stop=True)
            nc.vector.tensor_copy(gamb_s[:, :], gamb_ps[:, :])


        # ======== recurrence scope (reuses xT/Wg space) ========
        with tc.tile_pool(name="phA", bufs=2) as phA, \
             tc.tile_pool(name="invp", bufs=4) as invp, \
             tc.tile_pool(name="phB", bufs=3) as phB, \
             tc.tile_pool(name="outp", bufs=3) as outp, \
             tc.tile_pool(name="psA", bufs=4, space="PSUM") as psA, \
             tc.tile_pool(name="psB", bufs=2, space="PSUM") as psB, \
             tc.tile_pool(name="psO", bufs=2, space="PSUM") as psO:

            # ---- phase A: chunk-parallel precompute (j-major) ----
            XAK_t, Vb_t = {}, {}
            for j in range(NCH):
                for hl in range(HL):
                    rj = hl * NCH + j
                    sl = slice(j * 128, (j + 1) * 128)
                    kb = qk_all[:, 2 * hl, :]

                    gp = psA.tile([128, 256], F32, tag="psA", name=f"gp{rj}")
                    nc.tensor.matmul(gp[:, :].rearrange("p (b t) -> p b t", b=2),
                                     kb[:, sl], qk_all[:, 2 * hl:2 * hl + 2, sl],
                                     start=True, stop=True)

                    bcAN = psA.tile([128, 256], F32, tag="psA", name=f"bcAN{rj}")
                    nc.tensor.matmul(bcAN[:, :].rearrange("p (b t) -> p b t", b=2),
                                     ones_row[:, :],
                                     ccb_flat[:, :, rj * 128:(rj + 1) * 128],
                                     start=True, stop=True)
                    dAN = phA.tile([128, 256], F32, tag="dAN", bufs=4, name=f"dAN{rj}")
                    nc.vector.scalar_tensor_tensor(
                        out=dAN[:, :], in0=bcAN[:, :],
                        scalar=ccn_cols[:, rj:rj + 1],
                        in1=maskIS_s[:, :], op0=OP.subtract, op1=OP.add)
                    nc.scalar.activation(dAN[:, :], dAN[:, :], AF.Exp)

                    NT = phA.tile([128, 128], BF16, tag="NT", bufs=3, name=f"NT{rj}")
                    nc.vector.tensor_tensor(out=NT[:, :], in0=gp[:, 0:128],
                                            in1=dAN[:, 128:256], op=OP.mult)
                    AK = phA.tile([128, 256], BF16, tag="AK", bufs=4, name=f"AK{rj}")
                    nc.vector.tensor_tensor(out=AK[:, 0:128], in0=gp[:, 128:256],
                                            in1=dAN[:, 0:128], op=OP.mult)

                    ntp = psA.tile([128, 128], BF16, tag="psA", name=f"ntp{rj}")
                    nc.tensor.transpose(ntp[:, :], NT[:, :], identb_s[:, :])
                    Nb = invp.tile([128, 128], BF16, tag="Nb", name=f"Nb{rj}")
                    nc.vector.tensor_copy(Nb[:, :], ntp[:, :])

                    # T = (I-N)(I+N^2)(I+N^4) untransposed doubling: the
                    # result is directly the xak stationary (lhsT), so no
                    # final transpose is needed.
                    Ru = invp.tile([128, 128], BF16, tag="Rb", name=f"Ru{rj}")
                    nc.gpsimd.tensor_tensor(out=Ru[:, :], in0=identb_s[:, :],
                                            in1=Nb[:, :], op=OP.subtract)
                    sq2 = psA.tile([128, 256], F32, tag="psA", name=f"sq{rj}_2")
                    nc.tensor.matmul(sq2[:, 0:128], NT[:, :], Nb[:, :],
                                     start=True, stop=True)
                    nc.tensor.matmul(sq2[:, 128:256], Nb[:, :], NT[:, :],
                                     start=True, stop=True)
                    pair = invp.tile([128, 256], BF16, tag="pair",
                                     name=f"pr{rj}_2")
                    nc.scalar.activation(pair[:, :], sq2[:, :], AF.Copy)
                    rp2 = psA.tile([128, 128], F32, tag="psA", name=f"rp{rj}_2")
                    nc.tensor.matmul(rp2[:, :], pair[:, 128:256], Ru[:, :],
                                     start=True, stop=True)
                    Ru1 = invp.tile([128, 128], BF16, tag="Rb", name=f"Ru1{rj}")
                    nc.vector.tensor_tensor(out=Ru1[:, :], in0=Ru[:, :],
                                            in1=rp2[:, :], op=OP.add)
                    nt4 = psA.tile([128, 128], F32, tag="psA", name=f"nt4{rj}")
                    nc.tensor.matmul(nt4[:, :], pair[:, 0:128], pair[:, 128:256],
                                     start=True, stop=True)
                    nt4b = invp.tile([128, 128], BF16, tag="TmT_T",
                                     name=f"nt4b{rj}")
                    nc.scalar.activation(nt4b[:, :], nt4[:, :], AF.Copy)
                    rp4 = psA.tile([128, 128], F32, tag="psA", name=f"rp{rj}_4")
                    nc.tensor.matmul(rp4[:, :], nt4b[:, :], Ru1[:, :],
                                     start=True, stop=True)
                    Tst = phA.tile([128, 128], BF16, tag="TmT", bufs=3,
                                   name=f"T{rj}")
                    nc.vector.tensor_tensor(out=Tst[:, :], in0=Ru1[:, :],
                                            in1=rp4[:, :], op=OP.add)

                    vp = psA.tile([128, 256], BF16, tag="psA", name=f"vp{rj}")
                    nc.tensor.transpose(vp[:, 0:128], vT_all[:, 2 * hl, sl],
                                        identb_s[:, :])
                    nc.tensor.transpose(vp[:, 128:256], vT_all[:, 2 * hl + 1, sl],
                                        identb_s[:, :])
                    Vb = phA.tile([128, 256], BF16, tag="Vb", bufs=6, name=f"Vb{rj}")
                    nc.vector.tensor_scalar_mul(Vb[:, :], vp[:, :],
                                                b_cols[:, rj:rj + 1])
                    Vb_t[(hl, j)] = Vb

                    kp = psA.tile([128, 128], BF16, tag="psA", name=f"kp{rj}")
                    nc.tensor.transpose(kp[:, :], qk_all[:, 2 * hl, sl],
                                        identb_s[:, :])
                    nc.scalar.activation(AK[:, 128:256], kp[:, :], AF.Copy,
                                         scale=ed_cols[:, rj:rj + 1])

                    xak = psA.tile([128, 256], F32, tag="psA", name=f"xak{rj}")
                    nc.tensor.matmul(xak[:, :], Tst[:, :], AK[:, :],
                                     start=True, stop=True)
                    XAK = phA.tile([128, 256], BF16, tag="XAK", bufs=6,
                                   name=f"XAK{rj}")
                    nc.vector.tensor_copy(XAK[:, :], xak[:, :])
                    XAK_t[(hl, j)] = XAK

            # ---- phase B: sequential state recurrence ----
            for j in range(NCH):
                for hl in range(HL):
                    rj = hl * NCH + j
                    sl = slice(j * 128, (j + 1) * 128)
                    kn = qk_all[:, 2 * hl, :]
                    qn = qk_all[:, 2 * hl + 1, :]
                    XAK, Vb = XAK_t[(hl, j)], Vb_t[(hl, j)]

                    S_old = S_s[hl][(j - 1) % 2]
                    S_new = S_s[hl][j % 2]
                    pr1 = psB.tile([128, 2, 256], F32, tag="psB", name=f"pr1_{rj}")
                    pr2 = psB.tile([128, 2, 256], F32, tag="psB", name=f"pr2_{rj}")
                    if j == 0:
                        RHS = Vb
                    else:
                        wr = pr1[:, 0, :]
                        nc.tensor.matmul(wr, kn[:, sl], S_old[:, :],
                                         start=True, stop=True)
                        RHS = phB.tile([128, 256], BF16, tag="RHS", name=f"RHS{rj}")
                        nc.vector.scalar_tensor_tensor(
                            out=RHS[:, :], in0=wr,
                            scalar=nbl_cols[:, rj:rj + 1], in1=Vb[:, :],
                            op0=OP.mult, op1=OP.add)

                    kup = pr2[:, 0, :]
                    nc.tensor.matmul(kup, XAK[:, 128:256], RHS[:, :],
                                     start=True, stop=True)
                    if j == 0:
                        nc.vector.tensor_copy(S_new[:, :], kup)
                    else:
                        nc.vector.scalar_tensor_tensor(
                            out=S_new[:, :], in0=S_old[:, :],
                            scalar=gamb_s[:, rj:rj + 1], in1=kup,
                            op0=OP.mult, op1=OP.add)

                    t2 = pr2[:, 1, :]
                    nc.tensor.matmul(t2, XAK[:, 0:128], RHS[:, :],
                                     start=True, stop=True)
                    o_raw = og_s[:, j, hl * DV:(hl + 1) * DV]
                    if j == 0:
                        nc.scalar.activation(o_raw, t2, AF.Copy)
                    else:
                        t2s = phB.tile([128, 256], F32, tag="t2s", name=f"t2s{rj}")
                        nc.scalar.activation(t2s[:, :], t2, AF.Copy)
                        t1 = pr1[:, 1, :]
                        nc.tensor.matmul(t1, qn[:, sl], S_old[:, :],
                                         start=True, stop=True)
                        nc.vector.scalar_tensor_tensor(
                            out=o_raw, in0=t1, scalar=lam_cols[:, rj:rj + 1],
                            in1=t2s[:, :], op0=OP.mult, op1=OP.add)

                    # rmsnorm stats; sqrt batched per 4-chunk group below
                    osq = phB.tile([128, 256], F32, tag="osq", name=f"osq{rj}")
                    nc.scalar.activation(osq[:, :], o_raw, AF.Square,
                                         accum_out=ssq_all[:, j * HL + hl:j * HL + hl + 1])

            # ---- grouped rmsnorm scale + gate + o_proj ----
            for grp in range(4):
                c0, c1 = grp * 4, grp * 4 + 4
                nc.scalar.activation(rstd_all[:, c0:c1], ssq_all[:, c0:c1],
                                     AF.Sqrt, scale=1.0 / DV, bias=epsn_col[:, :])
                nc.vector.reciprocal(rstd_all[:, c0:c1], rstd_all[:, c0:c1])
                for tt in range(grp * 2, grp * 2 + 2):
                    for hl in range(HL):
                        cc = tt * HL + hl
                        nc.vector.scalar_tensor_tensor(
                            out=og_s[:, tt, hl * DV:(hl + 1) * DV],
                            in0=og_s[:, tt, hl * DV:(hl + 1) * DV],
                            scalar=rstd_all[:, cc:cc + 1],
                            in1=gvsnw[:, tt, hl * DV:(hl + 1) * DV],
                            op0=OP.mult, op1=OP.mult)
            for tt in range(NCH):
                otp = psO.tile([128, 512], BF16, tag="psO", name=f"otp{tt}")
                for cs in range(4):
                    nc.tensor.transpose(otp[:, cs * 128:(cs + 1) * 128],
                                        og_s[:, tt, cs * 128:(cs + 1) * 128],
                                        identb_s[:, :])
                oTt = outp.tile([128, 512], BF16, tag="oT", name=f"oT{tt}")
                nc.vector.tensor_copy(oTt[:, :], otp[:, :])
                ot_out = outp.tile([128, D], BF16, tag="oout", name=f"oo{tt}")
                ops2 = [psO.tile([128, 512], F32, tag="psO", name=f"op{tt}_{h}")
                        for h in range(2)]
                for cs in range(4):
                    for dh in range(2):
                        nc.tensor.matmul(ops2[dh][:, :],
                                         oTt[:, cs * 128:(cs + 1) * 128],
                                         Wo_s[:, cs, dh * 512:(dh + 1) * 512],
                                         start=(cs == 0), stop=(cs == 3))
                for dh in range(2):
                    nc.vector.tensor_copy(ot_out[:, dh * 512:(dh + 1) * 512],
                                          ops2[dh][:, :])
                nc.sync.dma_start(out=outD[tt * 128:(tt + 1) * 128, :],
                                  in_=ot_out[:, :])

    nc.compile()
    return nc


def _prep_core_inputs(inputs, core):
    b = core // 4
    hp = (core % 4) * 2
    bf = ml_dtypes.bfloat16
    x = np.asarray(inputs["x"], np.float32)
    Wq = np.asarray(inputs["Wq"], np.float32)
    Wk = np.asarray(inputs["Wk"], np.float32)
    Wv_f = np.asarray(inputs["Wv"], np.float32)
    Wg_f = np.asarray(inputs["Wg"], np.float32)
    Wb = np.asarray(inputs["Wb"], np.float32)
    Wa = np.asarray(inputs["Wa"], np.float32)
    Wo_f = np.asarray(inputs["Wo"], np.float32)
    conv_q = np.asarray(inputs["conv_q"], np.float32)
    conv_k = np.asarray(inputs["conv_k"], np.float32)
    conv_v = np.asarray(inputs["conv_v"], np.float32)
    A_log = np.asarray(inputs["A_log"], np.float32)
    dt_bias = np.asarray(inputs["dt_bias"], np.float32)
    norm_w = np.asarray(inputs["norm_w"], np.float32)

    h0, h1 = hp, hp + 1
    xTc = np.ascontiguousarray(x[b].T).astype(bf)
    Wqk_a = np.concatenate(
        [Wq[:, h0 * DK:(h0 + 1) * DK], Wq[:, h1 * DK:(h1 + 1) * DK],
         Wk[:, h0 * DK:(h0 + 1) * DK], Wk[:, h1 * DK:(h1 + 1) * DK]],
        axis=1).astype(bf)
    Wv_sh = np.ascontiguousarray(Wv_f[:, h0 * DV:(h0 + 2) * DV]).astype(bf)
    Wg_sh = np.ascontiguousarray(Wg_f[:, h0 * DV:(h0 + 2) * DV]).astype(bf)
    Wba_a = np.stack([Wb[:, h0], Wb[:, h1], Wa[:, h0], Wa[:, h1]],
                     axis=1).astype(bf)
    Wo_sh = np.ascontiguousarray(Wo_f[h0 * DV:(h0 + 2) * DV, :]
                                 * np.tile(norm_w, 2)[:, None]).astype(bf)

    convd_a = np.zeros((8, KC, 128, 128), bf)
    cblocks = [conv_q[h0 * DK:(h0 + 1) * DK], conv_q[h1 * DK:(h1 + 1) * DK],
               conv_k[h0 * DK:(h0 + 1) * DK], conv_k[h1 * DK:(h1 + 1) * DK],
               conv_v[h0 * DV:h0 * DV + 128], conv_v[h0 * DV + 128:(h0 + 1) * DV],
               conv_v[h1 * DV:h1 * DV + 128], conv_v[h1 * DV + 128:(h1 + 1) * DV]]
    ii = np.arange(128)
    for blk, w in enumerate(cblocks):
        for tap in range(KC):
            convd_a[blk, tap, ii, ii] = w[:, tap].astype(bf)

    adt_a = np.zeros((16, 2), np.float32)
    for hl in range(HL):
        adt_a[hl * NCH:(hl + 1) * NCH, 0] = -dt_bias[hp + hl]
        adt_a[hl * NCH:(hl + 1) * NCH, 1] = np.exp(A_log[hp + hl])

    tri = np.triu(np.ones((128, 128), bool))          # row i <= col t
    maskI_a = np.where(tri, 0.0, NEG).astype(np.float32)
    maskS_a = np.where(np.triu(np.ones((128, 128), bool), 1), 0.0,
                       NEG).astype(np.float32)
    ident = np.eye(128, dtype=np.float32)

    return {
        "xT": xTc, "Wqk": Wqk_a, "Wv": Wv_sh, "Wg": Wg_sh,
        "Wba": Wba_a, "Wo": Wo_sh, "convd": convd_a,
        "adt": adt_a, "maskI": maskI_a, "maskS": maskS_a,
        "identg": np.eye(16, dtype=np.float32),
        "identb": ident.astype(bf),
    }


def kernel(**inputs):
    if "nc" not in _cache:
        _cache["nc"] = build_kernel()
    nc = _cache["nc"]
    in_maps = [_prep_core_inputs(inputs, core) for core in range(8)]
    res = run_bass_kernel_spmd(nc, in_maps, core_ids=list(range(8)))
    out = np.zeros((B, T, D), np.float32)
    for b in range(B):
        for g in range(4):
            out[b] += res.results[4 * b + g]["out"].astype(np.float32)
    return out

